# revision 22
# baseline (speedup 1.0000x reference)
"""Multi-head self-attention (B=2, S=2048, E=1024, H=16) on 8 TRN2 cores.

Sharding: batch (2) x head-groups (4) -> 8 cores. Core c handles batch
c//4 and heads [4*(c%4), 4*(c%4)+4). Each core computes QKV projection,
attention, and its partial output projection; the host sums the 4
head-group partials per batch.

Device layout (per core, all matmul operands bf16, fp32 accumulation):
  - x arrives pre-transposed (E, S) so every projection contracts over
    partitions. Q,K are produced feature-major (qkT/kT: dh on partitions)
    which feeds scores^T = k @ q^T directly; V is produced token-major
    with an interleaved ones column so attn^T.T @ [v|1] yields both the
    attention output and the softmax row-sums in one accumulation.
  - softmax has no max-subtraction (scores are O(6) here; exp is safe in
    fp32) so exp(scale*x) is a single ACT pass over the score PSUM tile.
"""

from contextlib import ExitStack

import numpy as np
import ml_dtypes

import concourse.bass as bass
import concourse.tile as tile
from concourse import mybir
from concourse.vector_clock import ScopedClock
from concourse.bass_utils import run_bass_kernel_spmd

B, S, E = 2, 2048, 1024
H, DH = 16, 64
NCORES = 8
HL = 4              # heads per core
GF = HL * DH        # 256: local head feature dim
VW = DH + 1         # v block width incl. ones column
BF16 = mybir.dt.bfloat16
F32 = mybir.dt.float32
bf16 = ml_dtypes.bfloat16

P = 128
EK = E // P         # 8 contraction chunks
ST = S // P         # 16 token tiles
SQ = S // 512       # 4 query chunks for fp32-psum matmuls
SQ2 = S // 1024     # 2 query chunks for bf16-moving matmuls


def _split_excess_waits(nc):
    """Rewrite TPB instructions carrying >1 sem wait.

    This ISA build has a single (wait, update) event slot per 64B TPB
    instruction, but Tile emits instructions with several waits. Excess
    waits move onto same-engine NoOps inserted immediately before the
    instruction — the engine executes its stream in order, so waiting on
    preceding NoOps is equivalent. DMA instructions are exempt (their
    waits live in DGE descriptors, which support several).
    """
    for f in nc.m.functions:
        for bb in f.blocks:
            out = []
            for inst in bb.instructions:
                si = getattr(inst, "sync_info", None)
                waits = list(si.on_wait) if si and si.on_wait else []
                if len(waits) > 1:
                    ups = list(si.on_update) if si.on_update else []
                    assert len(ups) <= 1, f"{inst.name}: multi-update unsupported"
                    for w in waits[:-1]:
                        out.append(
                            mybir.InstNoOp(
                                name=f"I-{nc.next_id()}",
                                engine=inst.engine,
                                sync_info=mybir.SyncInfo(on_wait=[w], on_update=[]),
                                bass_nofuse=True,
                            )
                        )
                    inst.sync_info = mybir.SyncInfo(on_wait=[waits[-1]], on_update=ups)
                out.append(inst)
            bb.instructions[:] = out


class SafeTileContext(tile.TileContext):
    """TileContext whose tail drain splits sem waits across chained SP nops.

    This walrus build rejects >1 sync-wait command on a CTRL instruction;
    the stock tail drain can carry several and fails codegen ("Too many
    sync wait commands"). Semantics are unchanged: SP serially waits on
    every clock sem via nops, then drains and barriers as usual.
    """

    MAX_WAITS_PER_INST = 1

    def _drain_and_barrier(self, tick_clock, wait_clock):
        nc = self.nc
        probe = mybir.InstNoOp(
            name=nc.get_next_instruction_name(), engine=mybir.EngineType.SP
        )
        wait_clock.add_sem_waits(probe, ScopedClock({None: tick_clock.global_clock}))
        waits = list(probe.sync_info.on_wait) if probe.sync_info else []
        k = self.MAX_WAITS_PER_INST
        for i in range(0, len(waits), k):
            nop = nc.sync.nop(nofuse=True, hint="tail_wait")
            nop.ins.sync_info = mybir.SyncInfo(
                on_wait=list(waits[i : i + k]), on_update=[]
            )
        nc.sync.drain()
        nc.all_engine_barrier()
        popped = nc._tile_sem_poison_stack.pop()
        assert popped is self._sem_poison
        nc.clear_and_free_semaphores(list(self.sems.allocated().values()))
        nc.all_engine_barrier()


def _emit(ctx, tc, xt, wqk, wv, wo, y):
    nc = tc.nc
    rc_dram = nc.dram_tensor("rc_dram", [HL, S], F32)
    consts = ctx.enter_context(tc.tile_pool(name="consts", bufs=1))
    attn_pool = ctx.enter_context(tc.tile_pool(name="attn", bufs=6))
    scratch = ctx.enter_context(tc.tile_pool(name="scratch", bufs=8))
    ps_s_pool = ctx.enter_context(tc.tile_pool(name="ps_s", bufs=2, space="PSUM"))
    ps_acc_pool = ctx.enter_context(tc.tile_pool(name="ps_acc", bufs=4, space="PSUM"))

    # ---- PE warm-up: keep the HAM activity window busy while input DMAs
    # land so the first projection matmuls run at 2.4 GHz, not 1.2.
    warm = consts.tile([P, 64], BF16, name="warm")
    nc.vector.memset(warm, 0.0)
    ps_w = ps_acc_pool.tile([64, 64], F32, name="psw", tag="acc")
    for _ in range(110):
        nc.tensor.matmul(ps_w, warm, warm[:, 0:64], start=True, stop=True)

    # ---- stage inputs in SBUF (weights first; xt streams while QK runs)
    wqk_sb = []
    for e in range(EK):
        t = consts.tile([P, 2 * GF], BF16, name=f"wqk{e}")
        nc.sync.dma_start(out=t, in_=wqk[P * e : P * (e + 1), :])
        wqk_sb.append(t)
    wv_sb = []
    for e in range(EK):
        t = consts.tile([P, GF], BF16, name=f"wv{e}")
        nc.sync.dma_start(out=t, in_=wv[P * e : P * (e + 1), :])
        wv_sb.append(t)
    wo_sb = []
    for d in range(2):
        t = consts.tile([P, E], BF16, name=f"wo{d}")
        nc.sync.dma_start(out=t, in_=wo[P * d : P * (d + 1), :])
        wo_sb.append(t)

    xt_sb = []
    for e in range(EK):
        t = consts.tile([P, S], BF16, name=f"xt{e}")
        nc.sync.dma_start(out=t, in_=xt[P * e : P * (e + 1), :])
        xt_sb.append(t)
    # ---- QK projection -> feature-major qkT tiles
    # tile m: 0,1 = q heads (0,1),(2,3); 2,3 = k heads (0,1),(2,3)
    qkT_sb = []
    for m in range(4):
        t = consts.tile([P, S], BF16, name=f"qk{m}")
        qkT_sb.append(t)
    for m in range(4):
        for j in range(SQ):
            ps = ps_acc_pool.tile([P, 512], F32, name="psqk", tag="acc")
            for e in range(EK):
                nc.tensor.matmul(
                    ps,
                    wqk_sb[e][:, P * m : P * (m + 1)],
                    xt_sb[e][:, 512 * j : 512 * (j + 1)],
                    start=(e == 0),
                    stop=(e == EK - 1),
                )
            nc.vector.tensor_copy(qkT_sb[m][:, 512 * j : 512 * (j + 1)], ps)

    # ---- V projection -> token-major v tiles, ones column interleaved
    v_sb = []
    for it in range(ST):
        vt = consts.tile([P, HL * VW], BF16, name=f"v{it}")
        nc.vector.memset(vt, 1.0)
        v_sb.append(vt)
    for it in range(ST):
        ps = ps_acc_pool.tile([P, GF], F32, name="psv", tag="acc")
        for e in range(EK):
            nc.tensor.matmul(
                ps,
                xt_sb[e][:, P * it : P * (it + 1)],
                wv_sb[e],
                start=(e == 0),
                stop=(e == EK - 1),
            )
        dst = v_sb[it].rearrange("p (h c) -> p h c", c=VW)[:, :, 0:DH]
        src = ps.rearrange("p (h c) -> p h c", c=DH)
        nc.vector.tensor_copy(dst, src)

    # ---- attention, head-serial
    outT_sb = []
    for d in range(2):
        t = consts.tile([P, S], BF16, name=f"ot{d}")
        outT_sb.append(t)
    rb_sb = []
    for hh in range(HL):
        t = consts.tile([64, S], F32, name=f"rb{hh}")
        rb_sb.append(t)

    for h in range(HL):
        qt = qkT_sb[h // 2]
        kt = qkT_sb[2 + h // 2]
        po = 64 * (h % 2)
        d = h // 2
        vsl = slice(VW * h, VW * (h + 1))
        # four query-chunk accumulators live across the whole key loop so
        # each attn tile is consumed (4 AV matmuls) right after its exp —
        # keeps PE dense and lets scores(ik+1) overlap exp(ik).
        ps_os = []
        for j2 in range(SQ):
            t = ps_acc_pool.tile([VW, 512], F32, name=f"pso{j2}", tag="acc")
            ps_os.append(t)
        # scores+exp for tile ik overlap AV of tile ik-1 (one-step software
        # pipeline) so the in-order PE stream never queues an exp-dependent
        # AV matmul ahead of runnable score matmuls.
        prev = None
        for ik in range(ST + 1):
            halves = []
            if ik < ST:
                ksl = kt[po : po + 64, P * ik : P * (ik + 1)]
            for half in range(2):
                if ik < ST:
                    t_s = ps_s_pool.tile([P, 1024], F32, name="pss", tag="s")
                    for jj in range(2):
                        j = 2 * half + jj
                        nc.tensor.matmul(
                            t_s[:, 512 * jj : 512 * (jj + 1)],
                            ksl,
                            qt[po : po + 64, 512 * j : 512 * (j + 1)],
                            start=True,
                            stop=True,
                        )
                if prev is not None:
                    pat = prev[half]
                    for jj in range(2):
                        j = 2 * half + jj
                        nc.tensor.matmul(
                            ps_os[j],
                            v_sb[ik - 1][:, vsl],
                            pat[:, 512 * jj : 512 * (jj + 1)],
                            start=(ik == 1),
                            stop=(ik == ST),
                        )
                if ik < ST:
                    at = attn_pool.tile([P, 1024], BF16, name="at", tag="at")
                    nc.scalar.activation(
                        at,
                        t_s,
                        mybir.ActivationFunctionType.Exp,
                        scale=float(DH) ** -0.5,
                    )
                    halves.append(at)
            prev = halves if ik < ST else None
        # evacuate accumulators to SBUF at once (frees PSUM slots for the
        # next head); reciprocal + broadcast + normalize run off the
        # critical path from SBUF.
        for j2 in range(SQ):
            sl = slice(512 * j2, 512 * (j2 + 1))
            outU = scratch.tile([VW, 512], F32, name="ou", tag="ou")
            nc.vector.tensor_copy(outU, ps_os[j2])
            rc = scratch.tile([1, 512], F32, name="rc", tag="rc")
            nc.vector.reciprocal(rc, outU[DH : DH + 1, :])
            nc.sync.dma_start(out=rc_dram[h : h + 1, sl], in_=rc)
            nc.gpsimd.dma_start(
                out=rb_sb[h][:, sl],
                in_=rc_dram[h : h + 1, sl].partition_broadcast(64),
            )
            nc.gpsimd.tensor_mul(
                outT_sb[d][po : po + 64, sl],
                outU[0:DH, :],
                rb_sb[h][:, sl],
            )

    # ---- output projection, partial y = outT.T @ woT
    y_pool = ctx.enter_context(tc.tile_pool(name="ystage", bufs=3))
    for it in range(ST):
        y_sb = y_pool.tile([P, E], F32, name="ysb", tag="y")
        for u in range(2):
            ps_y = ps_acc_pool.tile([P, 512], F32, name="psy", tag="acc")
            for d in range(2):
                nc.tensor.matmul(
                    ps_y,
                    outT_sb[d][:, P * it : P * (it + 1)],
                    wo_sb[d][:, 512 * u : 512 * (u + 1)],
                    start=(d == 0),
                    stop=(d == 1),
                )
            if u == 0:
                nc.vector.tensor_copy(y_sb[:, 512 * u : 512 * (u + 1)], ps_y)
            else:
                nc.scalar.copy(y_sb[:, 512 * u : 512 * (u + 1)], ps_y)
        nc.sync.dma_start(out=y[P * it : P * (it + 1), :], in_=y_sb)


def build_nc(split_waits=True):
    nc = bass.Bass(trn_type="TRN2")
    xt = nc.dram_tensor("xt", [E, S], BF16, kind="ExternalInput")
    wqk = nc.dram_tensor("wqk", [E, 2 * GF], BF16, kind="ExternalInput")
    wv = nc.dram_tensor("wv", [E, GF], BF16, kind="ExternalInput")
    wo = nc.dram_tensor("wo", [GF, E], BF16, kind="ExternalInput")
    y = nc.dram_tensor("y", [S, E], F32, kind="ExternalOutput")
    with SafeTileContext(nc) as tc:
        with ExitStack() as ctx:
            _emit(ctx, tc, xt, wqk, wv, wo, y)
    if split_waits:
        _split_excess_waits(nc)
    return nc


_NC_CACHE = None


def _get_nc():
    global _NC_CACHE
    if _NC_CACHE is None:
        _NC_CACHE = build_nc()
    return _NC_CACHE


def make_in_maps(x, w_qkv, w_out):
    in_maps = []
    for c in range(NCORES):
        b, g = divmod(c, 4)
        q = w_qkv[GF * g : GF * (g + 1)]
        k = w_qkv[1024 + GF * g : 1024 + GF * (g + 1)]
        v = w_qkv[2048 + GF * g : 2048 + GF * (g + 1)]
        in_maps.append(
            {
                "xt": np.ascontiguousarray(np.asarray(x)[b].T).astype(bf16),
                "wqk": np.ascontiguousarray(
                    np.concatenate([q, k], axis=0).T
                ).astype(bf16),
                "wv": np.ascontiguousarray(np.asarray(v).T).astype(bf16),
                "wo": np.ascontiguousarray(
                    np.asarray(w_out)[:, GF * g : GF * (g + 1)].T
                ).astype(bf16),
            }
        )
    return in_maps


def gather_output(results):
    y = np.zeros((B, S, E), np.float32)
    for c in range(NCORES):
        y[c // 4] += results[c]["y"]
    return y


def kernel(x, w_qkv, w_out, **run_kwargs):
    nc = _get_nc()
    in_maps = make_in_maps(np.asarray(x), np.asarray(w_qkv), np.asarray(w_out))
    res = run_bass_kernel_spmd(nc, in_maps, core_ids=list(range(NCORES)), **run_kwargs)
    out = gather_output(res.results)
    if run_kwargs:
        kernel.last_results = res
    return out
